# revision 1
# baseline (speedup 1.0000x reference)
"""Trainium2 Bass kernel for nn_BrainBottleneckLocal (dense_cnn).

Sharding: spatial rows. H=16 rows are split 2-per-core across 8 NeuronCores;
every layer is then core-local (the LC weight is per-location, so the 604 MB
lc_w tensor splits 8x by row — the dominant DMA stream).

Per-core pipeline (free-dim layout is (h, w, n) everywhere):
  1. conv1x1 #1 + BN1 + ReLU on the core's 2 rows plus a 1-row halo each side
     (4 rows, boundary rows zero-padded by the host). Output is written
     straight to fp8-e4m3 (the LC input quantization).
  2. locally-connected 3x3 + BN2 + ReLU: fp8 weights (per-out-channel pow2
     scale folded in, undone by BN2's per-partition activation scale) and fp8
     patches via the tensor engine's DoubleRow perf mode (2 fp8 MACs/cell).
     lc_w streams as e4m3 — 18.9 MB per core instead of 37.7 (bf16).
  3. conv1x1 #2 + BN3, residual add (read from the bf16 conv1 input tile —
     no separate fp32 identity stream), ReLU -> resb bf16.
  4. opponent inhibition through a low-rank factorization of the mixing
     matrix g (host-side SVD; sigma ~ C/8 makes g numerically rank <~16):
     inh = A @ (B @ resb), then out = resb / (1 + inh), stored bf16.
Matmuls accumulate in fp32 PSUM. BN scales are folded into weights on the
host; BN biases apply via per-partition activation bias. All cores run an
identical program; only per-core data differs (boundary handling = zeroed
LC taps).
"""

import math
from contextlib import ExitStack

import numpy as np

import concourse.bacc as bacc
import concourse.bass as bass
import concourse.mybir as mybir
import concourse.tile as tile
from concourse.bass_utils import run_bass_kernel_spmd

F32 = mybir.dt.float32
BF16 = mybir.dt.bfloat16
FP8 = mybir.dt.float8e4
NPBF16 = mybir.dt.np(BF16)
NPFP8 = mybir.dt.np(FP8)

EPS = 1e-5
N, CIN, H, W = 64, 1024, 16, 16
WID, COUT = 256, 1024
NCORES = 8
RPC = H // NCORES          # rows per core = 2
HLO = RPC + 2              # rows incl halo = 4
WP = W + 2                 # padded width = 18
NLOC = RPC * W             # LC locations per core = 32
CC1 = CIN // 128           # 8
CCW = WID // 128           # 2
CC3 = COUT // 128          # 8
FR = RPC * W * N           # free size of per-core row block = 2048
RANK = 16                  # low-rank size for the inhibition mixing matrix
# packed bf16 consts width: w1t | w3t | gbt | 128x128 identity
CBF_X = CC1 * WID + CCW * COUT + CC3 * RANK + 128
AF = mybir.ActivationFunctionType
ALU = mybir.AluOpType
DR = mybir.MatmulPerfMode.DoubleRow


def _declare_drams(nc, variant):
    ap = {}
    ap["xb"] = nc.dram_tensor("xb", [CC1, 128, HLO * W * N], BF16,
                              kind="ExternalInput").ap()
    # grouped 4 locations per chunk -> 18.4KB DMA lines
    ap["lcw"] = nc.dram_tensor("lcw", [NLOC // 4, 128, 4 * 9 * 2 * WID], FP8,
                               kind="ExternalInput").ap()
    # packed constants: 1 DMA each instead of ~23 small ones
    #   cbf: [w1t (8*256) | w3t (2*1024) | gbt (8*16)] bf16, 128-part
    #   cf4: [b1 (2) | b2 (2) | s2 (2) | b3 (8)] f32 columns, 128-part
    ap["cbf"] = nc.dram_tensor("cbf", [128, CBF_X], BF16,
                               kind="ExternalInput").ap()
    ap["cf4"] = nc.dram_tensor("cf4", [128, 14], F32,
                               kind="ExternalInput").ap()
    if variant == "lr":
        # row RANK of ga is all-ones: stage-2 matmul then yields 1 + inh
        ap["ga"] = nc.dram_tensor("ga", [RANK + 1, COUT], BF16,
                                  kind="ExternalInput").ap()
    else:
        ap["gd"] = nc.dram_tensor("gd", [CC3, 128, COUT], BF16,
                                  kind="ExternalInput").ap()
    ap["ident"] = nc.dram_tensor("ident", [64, 64], BF16,
                                 kind="ExternalInput").ap()
    ap["out"] = nc.dram_tensor("out", [CC3, 128, FR], BF16,
                               kind="ExternalOutput").ap()
    return ap


ALL_STAGES = ("conv1", "lcdma", "lcmm", "conv3", "inhib")


def _build_nc(ktimes: int = 1, variant: str = "lr", stages=ALL_STAGES):
    nc = bacc.Bacc("TRN2", target_bir_lowering=False, debug=False,
                   num_devices=NCORES)
    ap = _declare_drams(nc, variant)
    with tile.TileContext(nc) as tc:
        if ktimes == 1:
            _trace_kernel(tc, nc, ap, variant, stages)
        else:
            with tc.For_i(0, ktimes, 1):
                _trace_kernel(tc, nc, ap, variant, stages)
    nc.compile()
    return nc


def _trace_kernel(tc, nc, ap, variant="lr", stages=ALL_STAGES):
    with ExitStack() as ctx:
        persist = ctx.enter_context(tc.tile_pool(name="persist", bufs=1))
        psum = ctx.enter_context(
            tc.tile_pool(name="psum", bufs=3, space="PSUM"))

        # ---- packed constants (4 DMAs total) ---------------------------
        cbf_t = persist.tile([128, CBF_X], BF16, name="cbf", tag="cbf")
        nc.sync.dma_start(out=cbf_t, in_=ap["cbf"])
        cf4_t = persist.tile([128, 14], F32, name="cf4", tag="cf4")
        nc.scalar.dma_start(out=cf4_t, in_=ap["cf4"])
        ident_t = persist.tile([64, 64], BF16, name="ident", tag="ident")
        nc.scalar.dma_start(out=ident_t, in_=ap["ident"])

        w1t_t = [cbf_t[:, cc * WID:(cc + 1) * WID] for cc in range(CC1)]
        off = CC1 * WID
        w3t_t = [cbf_t[:, off + oc * COUT:off + (oc + 1) * COUT]
                 for oc in range(CCW)]
        off += CCW * COUT
        gbt_t = [cbf_t[:, off + cc * RANK:off + (cc + 1) * RANK]
                 for cc in range(CC3)]
        i128_t = cbf_t[:, off + CC3 * RANK:off + CC3 * RANK + 128]
        b1_t = [cf4_t[:, c:c + 1] for c in range(CCW)]
        b2_t = [cf4_t[:, 2 + c:3 + c] for c in range(CCW)]
        s2_t = [cf4_t[:, 4 + c:5 + c] for c in range(CCW)]
        b3_t = [cf4_t[:, 6 + c:7 + c] for c in range(CC3)]
        if variant == "lr":
            ga_t = persist.tile([RANK + 1, COUT], BF16, name="ga", tag="ga")
            nc.scalar.dma_start(out=ga_t, in_=ap["ga"])
        else:
            gd_t = []
            for cc in range(CC3):
                t = persist.tile([128, COUT], BF16, name=f"gd_{cc}",
                                 tag=f"gd{cc}")
                nc.scalar.dma_start(out=t, in_=ap["gd"][cc])
                gd_t.append(t)

        out2_t = [persist.tile([128, FR], BF16, name=f"out2_{oc}",
                               tag=f"out2{oc}") for oc in range(CCW)]
        resb_t = [persist.tile([128, FR], BF16, name=f"resb_{oc}",
                               tag=f"resb{oc}") for oc in range(CC3)]

        # out1 fp8, padded width: [128, (c2, h4, w18, n64)], pad cols zeroed
        out1q = persist.tile([128, CCW * HLO * WP * N], FP8, name="out1q",
                             tag="out1q")
        o1v = out1q.rearrange("p (c h w n) -> p c h w n",
                              c=CCW, h=HLO, w=WP)
        # only the two W-pad columns need zeroing — conv1 overwrites the
        # rest every iteration. (A full-tile memset cost ~7us of gpsimd
        # and serialized against all conv1 activation writes.)
        nc.gpsimd.memset(o1v[:, :, :, 0, :], 0.0)
        nc.gpsimd.memset(o1v[:, :, :, W + 1, :], 0.0)

        lcw_pool = ctx.enter_context(tc.tile_pool(name="lcwp", bufs=3))
        lct_pool = ctx.enter_context(tc.tile_pool(name="lctp", bufs=3))
        div_pool = ctx.enter_context(tc.tile_pool(name="divp", bufs=6))
        yb_t = None
        if variant == "lr":
            # moving operand of inhibition stage 2; row RANK stays 1.0.
            # Double-buffered so stage 2 of slice ns doesn't block the
            # stage-1 copy of slice ns+1.
            yb_t = [persist.tile([RANK + 1, 512], BF16, name=f"yb{i}",
                                 tag=f"yb{i}") for i in range(2)]
            for t in yb_t:
                nc.gpsimd.memset(t, 1.0)

        # ---- phase 1: conv1x1 #1 + BN1 + ReLU on 4 halo rows -----------
        # xb stays alive through conv3's residual read (rows 1..2).
        # 2-row DMAs (4KB lines) split across two queues; rows 0-1 of all
        # chunks land first so conv1 can start early.
        xb_t = [persist.tile([128, HLO * W * N], BF16, name=f"xb_{cc}",
                             tag=f"xb{cc}") for cc in range(CC1)]
        # rows 0-1 of all chunks first so conv1 h=0,1 starts early
        for hh in range(2):
            for cc in range(CC1):
                eng = (nc.scalar, nc.gpsimd, nc.sync)[cc % 3]
                eng.dma_start(
                    out=xb_t[cc][:, hh * 2 * W * N:(hh + 1) * 2 * W * N],
                    in_=ap["xb"][cc][:, hh * 2 * W * N:(hh + 1) * 2 * W * N])

        for h in range(HLO if "conv1" in stages else 0):
            for oc in range(CCW):
                for q in range(2):
                    ps = psum.tile([128, 512], F32, name="ps1", tag="a",
                                   bufs=2)
                    base = h * (W * N) + q * 512
                    for cc in range(CC1):
                        nc.tensor.matmul(
                            ps,
                            w1t_t[cc][:, oc * 128:(oc + 1) * 128],
                            xb_t[cc][:, base:base + 512],
                            start=(cc == 0), stop=(cc == CC1 - 1))
                    # BN1 + ReLU into padded fp8 out1 (skip W-pad cols)
                    nc.scalar.activation(
                        out=o1v[:, oc, h, 1 + 8 * q:9 + 8 * q, :],
                        in_=ps.rearrange("p (w n) -> p w n", n=N),
                        func=AF.Relu, bias=b1_t[oc], scale=1.0)

        # ---- phase 2: locally-connected 3x3 + BN2 + ReLU (fp8 DR) ------
        # loc = hl*16 + j ; tap dk = di*3 + dj ; per tap one DoubleRow
        # matmul contracts 256 channels (2 per partition):
        #   patches [128, 2, 64] (stationary), lcw [128, 2, 256] (moving)
        #   -> psum [64n, 256o], PE-transposed back to [o, n] for BN2.
        if "lcmm" not in stages:
            for oc in range(CCW):
                nc.gpsimd.memset(out2_t[oc], 0.01)
        lw_shared = None
        if "lcdma" not in stages and "lcmm" in stages:
            lw_shared = persist.tile([128, 4 * 9 * 2 * WID], FP8,
                                     name="lw_shared", tag="lws")
            nc.gpsimd.memset(lw_shared, 0.01)
        for grp in range(NLOC // 4):
            if "lcdma" in stages:
                lw = lcw_pool.tile([128, 4 * 9 * 2 * WID], FP8,
                                   name="lcw_t", tag="lcw")
                # alternate the 18.9 MB stream across two DMA queues
                eng = (nc.sync, nc.gpsimd)[grp % 2]
                eng.dma_start(out=lw, in_=ap["lcw"][grp])
            else:
                lw = lw_shared
            if "lcmm" not in stages:
                continue
            lwv = lw.rearrange("p (l dk c o) -> p l dk c o", l=4, dk=9, c=2)
            # one psum tile holds both oc halves -> the 2-deep ring now
            # double-buffers across grps (transposes of grp g+1 overlap
            # BN2 activations of grp g) at the same bank cost
            pst_all = psum.tile([128, CCW * 4 * N], BF16, name="pst",
                                tag="tp", bufs=2)
            pst = [pst_all[:, oc * 4 * N:(oc + 1) * 4 * N]
                   for oc in range(CCW)]
            for li in range(4):
                loc = grp * 4 + li
                hl, j = divmod(loc, W)
                ps2 = psum.tile([64, WID], F32, name="ps2", tag="lc",
                                bufs=2)
                for dk in range(9):
                    di, dj = divmod(dk, 3)
                    nc.tensor.matmul(
                        ps2, o1v[:, :, hl + di, j + dj, :], lwv[:, li, dk],
                        start=(dk == 0), stop=(dk == 8), perf_mode=DR)
                tmpb = lct_pool.tile([64, WID], BF16, name="tmpb",
                                     tag="tmpb")
                nc.vector.tensor_copy(out=tmpb, in_=ps2)
                for oc in range(CCW):
                    nc.tensor.transpose(
                        pst[oc][:, li * N:(li + 1) * N],
                        tmpb[:, oc * 128:(oc + 1) * 128], ident_t)
            for oc in range(CCW):
                nc.scalar.activation(
                    out=out2_t[oc][:, grp * 4 * N:(grp + 1) * 4 * N],
                    in_=pst[oc], func=AF.Relu, bias=b2_t[oc],
                    scale=s2_t[oc])

        # ---- phase 3+4 per 512-slice of (hl,j,n): conv1x1 #2 + BN3 +
        # residual + ReLU, then inhibition + divide + store. ns-outer order
        # lets slice ns start as soon as LC produced grps 2ns..2ns+1.
        if "conv3" not in stages:
            for oc in range(CC3):
                nc.gpsimd.memset(resb_t[oc], 0.01)
        for ns in range(FR // 512):
            sl = slice(ns * 512, ns * 512 + 512)
            for oc3 in range(CC3 if "conv3" in stages else 0):
                ps = psum.tile([128, 512], F32, name="ps3", tag="a", bufs=2)
                for oc in range(CCW):
                    nc.tensor.matmul(
                        ps, w3t_t[oc][:, oc3 * 128:(oc3 + 1) * 128],
                        out2_t[oc][:, sl],
                        start=(oc == 0), stop=False)
                # residual add rides the PSUM accumulator: ps += I @ x
                res = xb_t[oc3][:, W * N + ns * 512:W * N + ns * 512 + 512]
                nc.tensor.matmul(ps, i128_t, res, start=False, stop=True)
                # resb = relu(ps + beta3)
                nc.scalar.activation(out=resb_t[oc3][:, sl], in_=ps,
                                     func=AF.Relu, bias=b3_t[oc3],
                                     scale=1.0)
            if "inhib" not in stages:
                continue
            if variant == "lr":
                yps = psum.tile([RANK, 512], F32, name="yps", tag="lc",
                                bufs=2)
                for cc in range(CC3):
                    nc.tensor.matmul(yps, gbt_t[cc], resb_t[cc][:, sl],
                                     start=(cc == 0), stop=(cc == CC3 - 1))
                yb = yb_t[ns % 2]
                nc.vector.tensor_copy(out=yb[:RANK], in_=yps)
            for oc in range(CC3):
                ps = psum.tile([128, 512], F32, name="ps4", tag="s4", bufs=2)
                if variant == "lr":
                    # lhsT row RANK is ones, yb row RANK is ones:
                    # psum = inh + 1 directly
                    nc.tensor.matmul(ps, ga_t[:, oc * 128:(oc + 1) * 128],
                                     yb, start=True, stop=True)
                    den = ps
                else:
                    for cc in range(CC3):
                        nc.tensor.matmul(
                            ps, gd_t[cc][:, oc * 128:(oc + 1) * 128],
                            resb_t[cc][:, sl],
                            start=(cc == 0), stop=(cc == CC3 - 1))
                    den = div_pool.tile([128, 512], F32, name="den",
                                        tag="den")
                    nc.scalar.add(out=den, in_=ps, add=1.0)
                rec = div_pool.tile([128, 512], F32, name="rec", tag="rec")
                # approx_fast: HW rel err measured identical to the exact
                # reciprocal (0.018229 both), and it is ~16us cheaper
                nc.vector.reciprocal_approx_fast(out=rec, in_=den)
                # final = resb * rec (resb is relu'd, so >= 0), in place.
                # Plain TensorTensor on gpsimd keeps it off the busy DVE
                # (walrus rejects TensorScalarPtr on Pool, but not this).
                nc.gpsimd.tensor_tensor(
                    out=resb_t[oc][:, sl], in0=resb_t[oc][:, sl],
                    in1=rec, op=ALU.mult)
                # half-row store (2KB lines) right behind each odd-slice
                # fin, so the final drain is minimal
                if ns % 2 == 1:
                    hs = slice((ns - 1) * 512, (ns + 1) * 512)
                    # SP/Act only: a store on the gpsimd queue would block
                    # the fin multiplies behind the ~1.8us transfer
                    eng = (nc.sync, nc.scalar)[oc % 2]
                    eng.dma_start(out=ap["out"][oc][:, hs],
                                  in_=resb_t[oc][:, hs])


def _pow2_scale(maxabs, target=120.0):
    return 2.0 ** np.floor(np.log2(target / np.maximum(maxabs, 1e-30)))


def _prep_inputs(x, w1, g1, b1, m1, v1, lc_w, g2, b2, m2, v2,
                 w3, g3, b3, m3, v3, sigmas):
    """Host-side shard + layout prep. Returns (variant, per-core maps)."""
    f4 = np.float32
    x = np.asarray(x, f4)
    inv1 = (g1 / np.sqrt(v1 + EPS)).astype(f4)
    beta1 = (b1 - m1 * inv1).astype(f4)
    inv2 = (g2 / np.sqrt(v2 + EPS)).astype(f4)
    beta2 = (b2 - m2 * inv2).astype(f4)
    inv3 = (g3 / np.sqrt(v3 + EPS)).astype(f4)
    beta3 = (b3 - m3 * inv3).astype(f4)

    w1t = (np.asarray(w1, f4) * inv1[:, None]).T.reshape(CC1, 128, WID)
    w1t = np.ascontiguousarray(w1t).astype(NPBF16)
    w3t = (np.asarray(w3, f4) * inv3[:, None]).T.reshape(CCW, 128, COUT)
    w3t = np.ascontiguousarray(w3t).astype(NPBF16)

    # lc_w: (1,O,C,H,W,9) -> fp8 [h, w, p, (dk, ch, o)] with c = ch*128+p,
    # scaled per out-channel to a power of 2 (undone by BN2's act scale).
    lcw = np.asarray(lc_w[0], f4) * inv2[:, None, None, None, None]
    s2m = _pow2_scale(np.abs(lcw).max(axis=(1, 2, 3, 4)))   # (O,)
    lcw *= s2m[:, None, None, None, None]
    lcw = lcw.transpose(2, 3, 1, 4, 0)             # (H, W, C, 9, O)
    lcw = lcw.reshape(H, W, CCW, 128, 9, WID)      # (h, w, ch, p, dk, o)
    lcw = lcw.transpose(0, 1, 3, 4, 2, 5)          # (h, w, p, dk, ch, o)
    lcw = np.clip(lcw, -240.0, 240.0)
    lcw = np.ascontiguousarray(lcw.reshape(H, W, 128, 9 * 2 * WID)).astype(NPFP8)
    s2inv = (1.0 / s2m).astype(f4)

    # x bf16: (C, Hpad, W, N), rows zero-padded at both ends
    xt = np.zeros((CIN, H + 2, W, N), f4)
    xt[:, 1:H + 1] = x.transpose(1, 2, 3, 0)
    xtb = xt.astype(NPBF16)

    # inhibition mixing matrix g on host (fp32), then SVD -> low rank
    idx = np.arange(COUT)
    ci = np.abs(idx + 1.0 - (COUT // 2 + 1.0))
    dist = ci[(idx[None, :] - idx[:, None]) % COUT]          # (O, C)
    sig = np.maximum(np.asarray(sigmas, np.float64), 0.5)
    g = np.exp(-dist.astype(np.float64) ** 2 / (2.0 * sig ** 2)) / sig
    g = g / g.sum(axis=0)                                     # (O, C)
    U, S, Vt = np.linalg.svd(g)
    tail = float(S[RANK] / S[0]) if S.shape[0] > RANK else 0.0
    variant = "lr" if tail < 1e-3 else "dense"
    if variant == "lr":
        A = (U[:, :RANK] * S[:RANK]).astype(f4)               # (O, r)
        B = Vt[:RANK].astype(f4)                              # (r, C)
        ga = np.concatenate([A.T, np.ones((1, COUT), f4)])    # (r+1, O)
        gbt = B.T.reshape(CC3, 128, RANK).astype(f4)          # (cc,p,r)
    else:
        gbt = np.zeros((CC3, 128, RANK), f4)

    # packed bf16 consts: [w1t | w3t | gbt | I128] along the free dim
    cbf = np.concatenate(
        [w1t.transpose(1, 0, 2).reshape(128, CC1 * WID).astype(f4),
         w3t.transpose(1, 0, 2).reshape(128, CCW * COUT).astype(f4),
         gbt.transpose(1, 0, 2).reshape(128, CC3 * RANK),
         np.eye(128, dtype=f4)],
        axis=1).astype(NPBF16)
    # packed f32 consts: [b1(2) b2(2) s2(2) b3(8)] as columns
    cf4 = np.concatenate(
        [beta1.reshape(CCW, 128).T, beta2.reshape(CCW, 128).T,
         s2inv.reshape(CCW, 128).T, beta3.reshape(CC3, 128).T],
        axis=1).astype(f4)
    com = {
        "ident": np.eye(64, dtype=NPBF16),
        "cbf": np.ascontiguousarray(cbf),
        "cf4": np.ascontiguousarray(cf4),
    }
    if variant == "lr":
        com["ga"] = np.ascontiguousarray(ga).astype(NPBF16)
    else:
        # device layout [c, o]: gd[cc][p, o] = g[o, cc*128+p]
        com["gd"] = np.ascontiguousarray(
            g.T.astype(f4).reshape(CC3, 128, COUT)).astype(NPBF16)

    in_maps = []
    for r in range(NCORES):
        r0 = r * RPC
        xbc = np.ascontiguousarray(xtb[:, r0:r0 + HLO]).reshape(
            CC1, 128, HLO * W * N)
        lw = np.ascontiguousarray(lcw[r0:r0 + RPC]).reshape(
            NLOC, 128, 9 * 2 * WID)
        if r == 0 or r == NCORES - 1:
            lw = lw.copy()
            if r == 0:           # row 0 locations: di=0 taps read row -1
                lw[0:W, :, 0:3 * 2 * WID] = 0
            if r == NCORES - 1:  # row 15 locations: di=2 taps read row 16
                lw[W:2 * W, :, 6 * 2 * WID:] = 0
        # group 4 locations per DMA chunk: [8, 128, 4*4608]
        lw = np.ascontiguousarray(
            lw.reshape(NLOC // 4, 4, 128, 9 * 2 * WID).transpose(0, 2, 1, 3)
        ).reshape(NLOC // 4, 128, 4 * 9 * 2 * WID)
        in_maps.append(dict(com, xb=xbc, lcw=lw))
    return variant, in_maps


def _assemble(results):
    """results: per-core dicts with 'out' [CC3,128,FR] bf16 -> (N,C,H,W)"""
    full = np.empty((N, COUT, H, W), np.float32)
    for r, res in enumerate(results):
        o = np.asarray(res["out"]).astype(np.float32)
        o = o.reshape(CC3, 128, RPC, W, N)
        # (cc, p, hl, j, n) -> (n, c, h, w)
        o = o.transpose(4, 0, 1, 2, 3).reshape(N, COUT, RPC, W)
        full[:, :, r * RPC:(r + 1) * RPC, :] = o
    return full


_NC_CACHE = {}


def get_nc(ktimes: int = 1, variant: str = "lr", stages=ALL_STAGES):
    key = (ktimes, variant, tuple(stages))
    if key not in _NC_CACHE:
        _NC_CACHE[key] = _build_nc(ktimes, variant, stages)
    return _NC_CACHE[key]


def kernel(**inputs):
    variant, in_maps = _prep_inputs(**inputs)
    nc = get_nc(1, variant)
    res = run_bass_kernel_spmd(nc, in_maps, core_ids=list(range(NCORES)))
    return _assemble(res.results)


if __name__ == "__main__":
    rng = np.random.default_rng(0)
    ins = {
        "x": rng.standard_normal((N, CIN, H, W)).astype(np.float32),
        "w1": (rng.standard_normal((WID, CIN)).astype(np.float32) * 0.05),
        "g1": rng.random(WID).astype(np.float32),
        "b1": rng.standard_normal(WID).astype(np.float32) * 0.05,
        "m1": np.zeros(WID, np.float32),
        "v1": np.ones(WID, np.float32),
        "lc_w": rng.standard_normal((1, WID, WID, H, W, 9)).astype(
            np.float32) * 0.05,
        "g2": rng.random(WID).astype(np.float32),
        "b2": rng.standard_normal(WID).astype(np.float32) * 0.05,
        "m2": np.zeros(WID, np.float32),
        "v2": np.ones(WID, np.float32),
        "w3": rng.standard_normal((COUT, WID)).astype(np.float32) * 0.05,
        "g3": rng.random(COUT).astype(np.float32),
        "b3": rng.standard_normal(COUT).astype(np.float32) * 0.05,
        "m3": np.zeros(COUT, np.float32),
        "v3": np.ones(COUT, np.float32),
        "sigmas": rng.random(COUT).astype(np.float32) + COUT / 8.0,
    }
    out = kernel(**ins)
    print("out", out.shape, out.dtype, float(np.abs(out).max()))

